# revision 3
# baseline (speedup 1.0000x reference)
"""GCN (4-layer GCNConv + BN + global mean/max pool + MLP) on 8 Trainium2 cores.

v2: balanced block layout + parity-split gather tables.
  - Nodes graph-contiguous per core, NO per-graph block padding: blocks are
    plain 128-windows of the core's node segment (B=ceil(nodes/128)), so a
    block can span (at most) 2 graphs.  Within each graph, nodes are ordered
    by a 2-D balancing serpentine on (deg_from_even_src, deg_from_odd_src)
    so each (block, parity) edge count is ~1024 +/- small -> K=9.
  - ONE u table per layer, AllGathered whole.  Gathers use elem_step=256
    (stride 2 rows) so even-src edges index j=src//2 into table[0:], odd-src
    edges into table[1:].  idx range NTOT/2 < 32768 fits int16.
  - Max pooling handles 2-graph blocks via maskA/maskB + gmaskA/gmaskB.
  - Everything else (one-hot dst mask matmul aggregation, BN stats AG,
    mean pooling, MLP) follows the v1 structure.
"""

import os
import numpy as np
import ml_dtypes

import concourse.bass as bass
import concourse.bacc as bacc
import concourse.tile as tile
from concourse import mybir
from concourse.masks import make_identity

P = 128
NC = 8
EPS = 1e-5
F32 = mybir.dt.float32
BF16 = mybir.dt.bfloat16
I16 = mybir.dt.int16
I32 = mybir.dt.int32
AF = mybir.ActivationFunctionType
ALU = mybir.AluOpType
BF16NP = ml_dtypes.bfloat16

PROFILE = False
SIM = False
LAST_RESULTS = {}


def _install_ntff_hook_shim():
    import sys
    import types
    import ctypes
    import contextlib
    if "antenv.axon_hooks" in sys.modules:
        return
    so = "/opt/axon/libaxon_pjrt.so"
    if not os.path.exists(so):
        return
    try:
        lib = ctypes.CDLL(so)
    except OSError:
        return
    if not hasattr(lib, "axon_start_nrt_profile"):
        return
    lib.axon_start_nrt_profile.argtypes = [
        ctypes.POINTER(ctypes.c_int64), ctypes.c_size_t]
    lib.axon_start_nrt_profile.restype = ctypes.c_int64
    lib.axon_stop_nrt_profile.argtypes = [ctypes.c_char_p]
    lib.axon_stop_nrt_profile.restype = ctypes.c_int64

    @contextlib.contextmanager
    def _hook(output_dir, device_ids):
        import jax
        jax.devices()
        if device_ids:
            ids = (ctypes.c_int64 * len(device_ids))(*device_ids)
            rc = lib.axon_start_nrt_profile(ids, len(device_ids))
        else:
            rc = lib.axon_start_nrt_profile(None, 0)
        if rc != 0:
            raise RuntimeError(f"axon_start_nrt_profile rc={rc}")
        try:
            yield
        finally:
            n = lib.axon_stop_nrt_profile(str(output_dir).encode())
            if n <= 0:
                print(f"ntff profile: {n} files written", flush=True)

    mod = types.ModuleType("antenv.axon_hooks")
    mod.get_axon_ntff_profile_hook = lambda: _hook
    mod.set_axon_ntff_profile_hook = lambda h: None
    sys.modules["antenv.axon_hooks"] = mod


# ----------------------------------------------------------------------------
# Host-side preprocessing
# ----------------------------------------------------------------------------
def _serpentine(ids, w_even, w_odd, nblk):
    """Order `ids` so that aligned 128-windows see balanced (w_even, w_odd)
    sums: interleave the (w_even - w_odd)-aware degree-sorted list as
    [0, n-1, 1, n-2, ...] so heavy and light nodes pair up and any
    128-window sum is ~128*mean +/- a few node degrees.
    """
    n = len(ids)
    order = np.argsort(-(w_even + w_odd), kind="stable")
    half = (n + 1) // 2
    inter = np.empty(n, dtype=np.int64)
    inter[0::2] = order[:half]
    inter[1::2] = order[half:][::-1]
    return ids[inter]


def _prep(x, edge_index, batch):
    N, F = x.shape
    E = edge_index.shape[1]
    G = 64
    GPC = (G + NC - 1) // NC

    batch = np.asarray(batch).astype(np.int64)
    src = np.asarray(edge_index[0]).astype(np.int64)
    dst = np.asarray(edge_index[1]).astype(np.int64)

    counts = np.bincount(batch, minlength=G)
    core_of_g = np.minimum(np.arange(G) // GPC, NC - 1)
    nodes_pc = np.array([counts[core_of_g == c].sum() for c in range(NC)])
    B = int((nodes_pc.max() + P - 1) // P)
    NODES_PC = B * P
    NTOT = NC * NODES_PC
    HROWS = NTOT // 2
    assert HROWS < 32768, f"idx range: {HROWS}"

    deg = np.bincount(dst, minlength=N).astype(np.int64)

    perm = np.argsort(batch, kind="stable")
    cum = np.concatenate([[0], np.cumsum(counts)])

    # --- two-pass layout: pass 1 rough (parity unknown), pass 2 balanced ---
    old2new = np.empty(N, dtype=np.int64)
    for _pass in range(2):
        if _pass == 0:
            w_even = deg.astype(np.float64)
            w_odd = np.zeros(N)
        else:
            # parity of src in current layout
            src_par = old2new[src] % 2
            w_even = np.bincount(dst[src_par == 0], minlength=N).astype(np.float64)
            w_odd = np.bincount(dst[src_par == 1], minlength=N).astype(np.float64)
        for c in range(NC):
            off = 0  # node offset within core segment
            base = c * NODES_PC
            for g in range(c * GPC, min((c + 1) * GPC, G)):
                n_g = int(counts[g])
                ids = perm[cum[g]:cum[g] + n_g]
                ids_o = _serpentine(ids, w_even[ids], w_odd[ids], 0)
                old2new[ids_o] = base + off + np.arange(n_g)
                off += n_g

    # --- per-node arrays in new layout ---
    x_new = np.zeros((NTOT, F), dtype=np.float32)
    x_new[old2new] = np.asarray(x, dtype=np.float32)
    deg_new = np.zeros(NTOT, dtype=np.float32)
    deg_new[old2new] = deg.astype(np.float32)
    bid_new = np.full(NTOT, -1.0, dtype=np.float32)
    bid_new[old2new] = batch.astype(np.float32)
    rm_new = np.zeros(NTOT, dtype=np.float32)
    rm_new[old2new] = 1.0

    def per_core_pb(a):  # [NTOT] -> [NC][128, B]
        return a.reshape(NC, B, P).transpose(0, 2, 1).copy()

    # --- pooling masks: block may span up to 2 graphs ---
    # gA[c,b] = graph of first real node in block, gB = second graph (or -1)
    bid_pb = bid_new.reshape(NC, B, P)
    maskA = np.zeros((NC, P, B), dtype=np.float32)
    maskB = np.zeros((NC, P, B), dtype=np.float32)
    gmaskA = np.zeros((NC, 1, G * B), dtype=np.float32)
    gmaskB = np.zeros((NC, 1, G * B), dtype=np.float32)
    for c in range(NC):
        for b in range(B):
            col = bid_pb[c, b]  # [P] graph ids (or -1 pad)
            real = col >= 0
            if not real.any():
                continue
            gs = np.unique(col[real]).astype(np.int64)
            assert len(gs) <= 2, f"block spans {len(gs)} graphs"
            gA = int(gs[0])
            maskA[c, :, b] = (col == gA).astype(np.float32)
            gmaskA[c, 0, gA * B + b] = 1.0
            if len(gs) == 2:
                gB = int(gs[1])
                maskB[c, :, b] = (col == gB).astype(np.float32)
                gmaskB[c, 0, gB * B + b] = 1.0

    # --- edges: key = (dst core, dst block, src parity) ---
    src_n = old2new[src]
    dst_n = old2new[dst]
    core_e = dst_n // NODES_PC
    lb = (dst_n % NODES_PC) // P
    dl = dst_n % P
    par = src_n % 2
    idxv = src_n // 2  # row in the stride-2 table view

    key = (core_e * B + lb) * 2 + par
    order = np.argsort(key, kind="stable")
    key_s = key[order]
    idx_s = idxv[order]
    dl_s = dl[order]
    grp_cnt = np.bincount(key_s, minlength=NC * B * 2)
    K = int((grp_cnt.max() + P - 1) // P)
    CAP = K * P

    idx_pad = np.zeros((NC * B * 2, CAP), dtype=np.int16)
    dl_pad = np.full((NC * B * 2, CAP), -1.0, dtype=np.float32)
    starts = np.concatenate([[0], np.cumsum(grp_cnt)])
    flat_pos = (np.arange(E) - starts[key_s]) + key_s * CAP
    idx_pad.reshape(-1)[flat_pos] = idx_s.astype(np.int16)
    dl_pad.reshape(-1)[flat_pos] = dl_s.astype(np.float32)

    idx_pad = idx_pad.reshape(NC, B, 2, CAP)
    dl_pad = dl_pad.reshape(NC, B, 2, CAP)

    # uniform (across cores) real-count per (block,parity), 16-aligned; the
    # gather processes only the first reg entries (trailing idx=-1 skipped).
    cnt_cbp = grp_cnt.reshape(NC, B, 2)
    regs = cnt_cbp.max(axis=0)                      # [B, 2]
    regs = ((regs + 15) // 16) * 16
    regs = np.minimum(regs, CAP).astype(np.int64)
    for b in range(B):
        for h in range(2):
            idx_pad[:, b, h, regs[b, h]:] = -1

    def wrap_idx(stream):  # [L] int16 -> [128, L//16]
        L = stream.shape[0]
        w = stream.reshape(L // 16, 16).T
        return np.tile(w, (8, 1)).copy()

    idx_w = np.empty((NC, 2, P, B * CAP // 16), dtype=np.int16)
    dl_t = np.empty((NC, 2, P, B * K), dtype=BF16NP)
    for c in range(NC):
        for h in range(2):
            stream = idx_pad[c, :, h, :].reshape(-1)
            idx_w[c, h] = wrap_idx(stream)
            dl_t[c, h] = dl_pad[c, :, h, :].reshape(B * K, P).T.astype(BF16NP)

    cfg = dict(N=N, F=F, G=G, B=B, K=K, NODES_PC=NODES_PC, NTOT=NTOT,
               regs=regs)
    percore = dict(
        x=[x_new.reshape(NC, NODES_PC, F)[c].astype(BF16NP) for c in range(NC)],
        deg=list(per_core_pb(deg_new)),
        batchid=list(per_core_pb(bid_new)),
        realmask=list(per_core_pb(rm_new)),
        maskA=[maskA[c] for c in range(NC)],
        maskB=[maskB[c] for c in range(NC)],
        gmaskA=[gmaskA[c] for c in range(NC)],
        gmaskB=[gmaskB[c] for c in range(NC)],
        idx_ev=[idx_w[c, 0] for c in range(NC)],
        idx_od=[idx_w[c, 1] for c in range(NC)],
        dstl_ev=[dl_t[c, 0] for c in range(NC)],
        dstl_od=[dl_t[c, 1] for c in range(NC)],
    )
    return cfg, percore


# ----------------------------------------------------------------------------
# Bass program
# ----------------------------------------------------------------------------
def _build(cfg, GRP=1):
    B, K, G = cfg["B"], cfg["K"], cfg["G"]
    REGS = cfg["regs"]  # [B, 2] real idx counts per (block, parity)
    NODES_PC, NTOT = cfg["NODES_PC"], cfg["NTOT"]
    F = cfg["F"]
    NREAL = cfg["N"]
    CAP = K * P
    L = B * CAP

    nc = bacc.Bacc("TRN2", target_bir_lowering=False, debug=False,
                   num_devices=NC, num_swdge_queues=4)

    din = {}
    def dram_in(name, shape, dt):
        din[name] = nc.dram_tensor(name, shape, dt, kind="ExternalInput")
        return din[name]

    x_d = dram_in("x", [NODES_PC, F], BF16)
    deg_d = dram_in("deg", [P, B], F32)
    bid_d = dram_in("batchid", [P, B], F32)
    rm_d = dram_in("realmask", [P, B], F32)
    mA_d = dram_in("maskA", [P, B], F32)
    mB_d = dram_in("maskB", [P, B], F32)
    gmA_d = dram_in("gmaskA", [1, G * B], F32)
    gmB_d = dram_in("gmaskB", [1, G * B], F32)
    ixev_d = dram_in("idx_ev", [P, L // 16], I16)
    ixod_d = dram_in("idx_od", [P, L // 16], I16)
    dlev_d = dram_in("dstl_ev", [P, B * K], BF16)
    dlod_d = dram_in("dstl_od", [P, B * K], BF16)
    W_d = dram_in("W", [4, F, F], BF16)
    lw1_d = dram_in("lw1", [2 * F, F], F32)
    lw2_d = dram_in("lw2", [F, 16], F32)
    b4_d = dram_in("b4", [1, F], F32)
    gam_d = dram_in("gamma", [3, F], F32)
    bet_d = dram_in("beta", [3, F], F32)
    lb1_d = dram_in("lb1", [1, F], F32)
    lb2_d = dram_in("lb2", [1, 16], F32)
    out_d = nc.dram_tensor("out", [64, 16], F32, kind="ExternalOutput")

    NGRP = (B + GRP - 1) // GRP

    with tile.TileContext(nc) as tc:
        with (
            tc.tile_pool(name="dram", bufs=1, space="DRAM") as dram,
            tc.tile_pool(name="persist", bufs=1) as ps,
            tc.tile_pool(name="work", bufs=2) as wk,
            tc.tile_pool(name="hTp", bufs=3) as hTp,
            tc.tile_pool(name="gbufp", bufs=3) as gbufp,
            tc.tile_pool(name="mbufp", bufs=3) as mbufp,
            tc.tile_pool(name="ps_tr", bufs=1, space="PSUM") as ps_tr,
            tc.tile_pool(name="ps_t", bufs=1, space="PSUM") as ps_t,
            tc.tile_pool(name="ps_agg", bufs=3, space="PSUM") as ps_agg,
            tc.tile_pool(name="ps_stats", bufs=1, space="PSUM") as ps_stats,
            tc.tile_pool(name="ps_pool", bufs=1, space="PSUM") as ps_pool,
            tc.tile_pool(name="ps_misc", bufs=1, space="PSUM") as ps_misc,
        ):
            # --- internal DRAM ---
            u_stage = dram.tile([NODES_PC, F], BF16)
            tables = [dram.tile([NTOT, F], BF16, addr_space="Shared",
                                name=f"tbl{j}", tag=f"tbl{j}")
                      for j in range(4)]
            stats_in = dram.tile([1, 2 * F], F32)
            stats_outs = [dram.tile([NC, 2 * F], F32, addr_space="Shared",
                                    name=f"stats_out{j}", tag=f"stats_out{j}")
                          for j in range(3)]
            pool_in = dram.tile([64, 2 * F + 1], F32)
            pool_out = dram.tile([NC * 64, 2 * F + 1], F32, addr_space="Shared")

            # --- persistent SBUF ---
            ident_b = ps.tile([P, P], BF16)
            make_identity(nc, ident_b[:])
            ident_f = ps.tile([P, P], F32)
            make_identity(nc, ident_f[:])
            iota_i = ps.tile([P, P], I32)
            nc.gpsimd.iota(iota_i[:], pattern=[[1, P]], base=0,
                           channel_multiplier=0)
            iota_b = ps.tile([P, P], BF16)
            nc.vector.tensor_copy(iota_b[:], iota_i[:])
            giota_i = ps.tile([P, 64], I32)
            nc.gpsimd.iota(giota_i[:], pattern=[[1, 64]], base=0,
                           channel_multiplier=0)
            giota_f = ps.tile([P, 64], F32)
            nc.vector.tensor_copy(giota_f[:], giota_i[:])
            ones_row = ps.tile([1, P], F32)
            nc.vector.memset(ones_row[:], 1.0)
            ones8 = ps.tile([NC, 1], F32)
            nc.vector.memset(ones8[:], 1.0)

            deg_sb = ps.tile([P, B], F32)
            nc.sync.dma_start(out=deg_sb[:], in_=deg_d[:, :])
            dinv = ps.tile([P, B], F32)
            nc.vector.tensor_scalar_add(out=dinv[:], in0=deg_sb[:], scalar1=1.0)
            nc.vector.reciprocal(out=dinv[:], in_=dinv[:])
            nc.scalar.activation(dinv[:], dinv[:], AF.Sqrt)
            bid_sb = ps.tile([P, B], F32)
            nc.sync.dma_start(out=bid_sb[:], in_=bid_d[:, :])
            rm_sb = ps.tile([P, B], F32)
            nc.sync.dma_start(out=rm_sb[:], in_=rm_d[:, :])
            mA_sb = ps.tile([P, B], F32)
            nc.sync.dma_start(out=mA_sb[:], in_=mA_d[:, :])
            mB_sb = ps.tile([P, B], F32)
            nc.sync.dma_start(out=mB_sb[:], in_=mB_d[:, :])
            gmA_sb = ps.tile([1, G * B], F32)
            nc.sync.dma_start(out=gmA_sb[:], in_=gmA_d[:, :])
            gmB_sb = ps.tile([1, G * B], F32)
            nc.sync.dma_start(out=gmB_sb[:], in_=gmB_d[:, :])
            ixev_sb = ps.tile([P, L // 16], I16)
            nc.sync.dma_start(out=ixev_sb[:], in_=ixev_d[:, :])
            ixod_sb = ps.tile([P, L // 16], I16)
            nc.sync.dma_start(out=ixod_sb[:], in_=ixod_d[:, :])
            dlev_sb = ps.tile([P, B * K], BF16)
            nc.sync.dma_start(out=dlev_sb[:], in_=dlev_d[:, :])
            dlod_sb = ps.tile([P, B * K], BF16)
            nc.sync.dma_start(out=dlod_sb[:], in_=dlod_d[:, :])
            b4row = ps.tile([1, F], F32)
            nc.sync.dma_start(out=b4row[:], in_=b4_d[:, :])
            lb1row = ps.tile([1, F], F32)
            nc.sync.dma_start(out=lb1row[:], in_=lb1_d[:, :])
            lb2row = ps.tile([1, 16], F32)
            nc.sync.dma_start(out=lb2row[:], in_=lb2_d[:, :])
            gam_sb = ps.tile([1, 3 * F], F32)
            bet_sb = ps.tile([1, 3 * F], F32)
            for j in range(3):
                nc.sync.dma_start(out=gam_sb[:, j * F:(j + 1) * F],
                                  in_=gam_d[j:j + 1, :])
                nc.sync.dma_start(out=bet_sb[:, j * F:(j + 1) * F],
                                  in_=bet_d[j:j + 1, :])

            h_sb = ps.tile([P, B * F], BF16)
            nc.sync.dma_start(
                out=h_sb[:].rearrange("p (b f) -> p b f", b=B),
                in_=x_d[:, :].rearrange("(b p) f -> p b f", p=P),
            )
            conv_sb = ps.tile([P, B * F], F32)
            u_sb = ps.tile([P, B * F], BF16)

            b4_ps = ps_misc.tile([P, 512], F32, tag="misc")
            nc.tensor.matmul(b4_ps[:, 0:F], lhsT=ones_row[:], rhs=b4row[:],
                             start=True, stop=True)
            b4_bc = ps.tile([P, F], F32)
            nc.scalar.activation(b4_bc[:], b4_ps[:, 0:F], AF.Copy)

            qi = 0  # running SWDGE queue counter (queue = qi % 4 always)
            cf_prev = None  # BN coef of previous layer, applied in phase A

            # ---------------- layers ----------------
            for k in range(4):
                W_sb = wk.tile([P, F], BF16, tag="W")
                nc.sync.dma_start(out=W_sb[:], in_=W_d[k, :, :])

                # phase A: (BN+relu of prev layer per block, then)
                # u = dinv * (h @ W) per block -> u_sb + u_stage
                for b in range(B):
                    if cf_prev is not None:
                        nc.vector.tensor_tensor(
                            out=conv_sb[:, b * F:(b + 1) * F],
                            in0=conv_sb[:, b * F:(b + 1) * F],
                            in1=cf_prev[:, 0:F], op=ALU.mult)
                        nc.vector.tensor_tensor(
                            out=conv_sb[:, b * F:(b + 1) * F],
                            in0=conv_sb[:, b * F:(b + 1) * F],
                            in1=cf_prev[:, F:2 * F], op=ALU.add)
                        nc.scalar.activation(h_sb[:, b * F:(b + 1) * F],
                                             conv_sb[:, b * F:(b + 1) * F],
                                             AF.Relu)
                    hT_ps = ps_tr.tile([P, P], BF16, tag="trf")
                    nc.tensor.transpose(hT_ps[:], h_sb[:, b * F:(b + 1) * F],
                                        ident_b[:])
                    hT = hTp.tile([P, P], BF16, tag="hT")
                    nc.scalar.activation(hT[:], hT_ps[:], AF.Copy)
                    t_ps = ps_t.tile([P, F], F32, tag="tps")
                    nc.tensor.matmul(t_ps[:], lhsT=hT[:], rhs=W_sb[:],
                                     start=True, stop=True)
                    nc.vector.tensor_scalar_mul(
                        out=u_sb[:, b * F:(b + 1) * F], in0=t_ps[:],
                        scalar1=dinv[:, b:b + 1])
                    nc.sync.dma_start(out=u_stage[b * P:(b + 1) * P, :],
                                      in_=u_sb[:, b * F:(b + 1) * F])
                nc.gpsimd.collective_compute(
                    "AllGather", ALU.bypass,
                    replica_groups=[list(range(NC))],
                    ins=[u_stage[:, :].opt()],
                    outs=[tables[k][:, :].opt()],
                )

                # phase C: aggregate.  even pass = partial, odd pass = finish.
                table = tables[k]
                if k < 3:
                    st_mu = ps_stats.tile([1, F], F32, tag="stats")
                    st_e2 = ps_pool.tile([1, F], F32, tag="pool")
                tblv = table[:, :].rearrange("(n two) f -> n (two f)", two=2)
                for half in range(2):
                    ix_sb = ixev_sb if half == 0 else ixod_sb
                    dl_sb = dlev_sb if half == 0 else dlod_sb
                    # even rows: pair-row cols [0:F); odd rows: cols [F:2F)
                    tbl_ap = (tblv[:, 0:F] if half == 0
                              else tblv[:, F:2 * F])
                    for g in range(NGRP):
                        b0 = g * GRP
                        nb = min(GRP, B - b0)
                        nidx = nb * CAP
                        nreal = int(sum(REGS[b0 + i][half]
                                        for i in range(nb)))
                        gt = gbufp.tile([P, nb * K, P], BF16, tag="g")
                        j0 = nreal // P
                        if j0 < nb * K:
                            # skipped tail slots are never written by the
                            # gather; clear them so mask*garbage can't NaN
                            nc.vector.memset(gt[:, j0:nb * K, :], 0.0)
                        nc.gpsimd.dma_gather(
                            out_ap=gt[:], in_ap=tbl_ap,
                            idxs_ap=ix_sb[:, b0 * CAP // 16:(b0 * CAP + nidx) // 16],
                            num_idxs=nidx, num_idxs_reg=nreal, elem_size=F,
                            elem_step=2 * F,
                            single_packet=False, queue_num=qi % 4)
                        qi += 1
                        mt = mbufp.tile([P, nb * K, P], BF16, tag="m")
                        nc.vector.tensor_tensor(
                            out=mt[:], in0=dl_sb[:, b0 * K:(b0 + nb) * K]
                            .to_broadcast([P, nb * K, P]),
                            in1=iota_b[:].rearrange("p (o f) -> p o f", o=1)
                            .to_broadcast([P, nb * K, P]),
                            op=ALU.is_equal)
                        for bb in range(nb):
                            b = b0 + bb
                            agg = ps_agg.tile([P, F], F32, tag="agg")
                            for j in range(K):
                                nc.tensor.matmul(
                                    agg[:], lhsT=mt[:, bb * K + j, :],
                                    rhs=gt[:, bb * K + j, :],
                                    start=(j == 0),
                                    stop=(half == 0 and j == K - 1))
                            if half == 0:
                                nc.scalar.activation(
                                    conv_sb[:, b * F:(b + 1) * F], agg[:],
                                    AF.Copy)
                            else:
                                nc.tensor.matmul(
                                    agg[:], lhsT=ident_b[:],
                                    rhs=u_sb[:, b * F:(b + 1) * F],
                                    start=False, stop=True)
                                nc.vector.tensor_tensor(
                                    out=conv_sb[:, b * F:(b + 1) * F],
                                    in0=conv_sb[:, b * F:(b + 1) * F],
                                    in1=agg[:], op=ALU.add)
                                nc.vector.tensor_scalar_mul(
                                    out=conv_sb[:, b * F:(b + 1) * F],
                                    in0=conv_sb[:, b * F:(b + 1) * F],
                                    scalar1=dinv[:, b:b + 1])
                                if k < 3:
                                    sq = wk.tile([P, F], F32, tag="sq")
                                    nc.vector.tensor_tensor(
                                        out=sq[:],
                                        in0=conv_sb[:, b * F:(b + 1) * F],
                                        in1=conv_sb[:, b * F:(b + 1) * F],
                                        op=ALU.mult)
                                    nc.tensor.matmul(
                                        st_mu[:, :], lhsT=rm_sb[:, b:b + 1],
                                        rhs=conv_sb[:, b * F:(b + 1) * F],
                                        start=(b == 0), stop=(b == B - 1),
                                        skip_group_check=True)
                                    nc.tensor.matmul(
                                        st_e2[:, :],
                                        lhsT=rm_sb[:, b:b + 1], rhs=sq[:],
                                        start=(b == 0), stop=(b == B - 1),
                                        skip_group_check=True)

                if k < 3:
                    # BN stats allreduce (via allgather) + apply + relu
                    st_sb = wk.tile([1, 2 * F], F32, tag="strow")
                    nc.scalar.activation(st_sb[:, 0:F], st_mu[:, :], AF.Copy)
                    nc.scalar.activation(st_sb[:, F:2 * F], st_e2[:, :], AF.Copy)
                    nc.sync.dma_start(out=stats_in[:, :], in_=st_sb[:])
                    stats_out = stats_outs[k]
                    nc.gpsimd.collective_compute(
                        "AllGather", ALU.bypass,
                        replica_groups=[list(range(NC))],
                        ins=[stats_in[:, :].opt()],
                        outs=[stats_out[:, :].opt()],
                    )
                    srows = wk.tile([NC, 2 * F], F32, tag="srows")
                    nc.sync.dma_start(out=srows[:], in_=stats_out[:, :])
                    tot_ps = ps_misc.tile([P, 512], F32, tag="misc")
                    nc.tensor.matmul(tot_ps[0:1, 0:2 * F], lhsT=ones8[:],
                                     rhs=srows[:], start=True, stop=True)
                    mrow = wk.tile([1, 2 * F], F32, tag="mrow")
                    nc.scalar.activation(mrow[:], tot_ps[0:1, 0:2 * F],
                                         AF.Copy, scale=1.0 / NREAL)
                    coef = wk.tile([1, 2 * F], F32, tag="coef")
                    nc.vector.tensor_tensor(out=coef[:, 0:F],
                                            in0=mrow[:, 0:F], in1=mrow[:, 0:F],
                                            op=ALU.mult)
                    nc.vector.tensor_tensor(out=coef[:, 0:F],
                                            in0=mrow[:, F:2 * F],
                                            in1=coef[:, 0:F], op=ALU.subtract)
                    nc.vector.tensor_scalar_add(out=coef[:, 0:F],
                                                in0=coef[:, 0:F], scalar1=EPS)
                    nc.vector.reciprocal(out=coef[:, 0:F], in_=coef[:, 0:F])
                    nc.scalar.activation(coef[:, 0:F], coef[:, 0:F], AF.Sqrt)
                    nc.vector.tensor_tensor(out=coef[:, 0:F], in0=coef[:, 0:F],
                                            in1=gam_sb[:, k * F:(k + 1) * F],
                                            op=ALU.mult)
                    tmp = wk.tile([1, F], F32, tag="tmprow")
                    nc.vector.tensor_tensor(out=tmp[:], in0=mrow[:, 0:F],
                                            in1=coef[:, 0:F], op=ALU.mult)
                    nc.vector.tensor_tensor(out=coef[:, F:2 * F],
                                            in0=bet_sb[:, k * F:(k + 1) * F],
                                            in1=tmp[:], op=ALU.subtract)
                    cf_ps = ps_misc.tile([P, 512], F32, tag="misc")
                    nc.tensor.matmul(cf_ps[:, 0:2 * F], lhsT=ones_row[:],
                                     rhs=coef[:], start=True, stop=True)
                    cf_bc = ps.tile([P, 2 * F], F32)
                    nc.scalar.activation(cf_bc[:], cf_ps[:, 0:2 * F], AF.Copy)
                    cf_prev = cf_bc  # applied per-block in next phase A
                else:
                    nc.vector.tensor_tensor(
                        out=conv_sb[:], in0=conv_sb[:],
                        in1=b4_bc[:].rearrange("p (o f) -> p o f", o=1)
                        .to_broadcast([P, B, F]), op=ALU.add)
                    nc.scalar.activation(conv_sb[:], conv_sb[:], AF.Relu)

            # ---------------- pooling ----------------
            bmaxA = ps.tile([P, B], F32)
            bmaxB = ps.tile([P, B], F32)
            pool_s = ps_pool.tile([64, F], F32, tag="pool")
            pool_c = ps_stats.tile([64, 1], F32, tag="stats")
            for b in range(B):
                nc.vector.tensor_scalar_mul(
                    out=conv_sb[:, b * F:(b + 1) * F],
                    in0=conv_sb[:, b * F:(b + 1) * F],
                    scalar1=rm_sb[:, b:b + 1])
                S = wk.tile([P, 64], F32, tag="S")
                nc.vector.tensor_tensor(
                    out=S[:], in0=bid_sb[:, b:b + 1].to_broadcast([P, 64]),
                    in1=giota_f[:], op=ALU.is_equal)
                nc.tensor.matmul(pool_s[:, :], lhsT=S[:],
                                 rhs=conv_sb[:, b * F:(b + 1) * F],
                                 start=(b == 0), stop=(b == B - 1),
                                 skip_group_check=True)
                nc.tensor.matmul(pool_c[:, :], lhsT=S[:],
                                 rhs=rm_sb[:, b:b + 1],
                                 start=(b == 0), stop=(b == B - 1),
                                 skip_group_check=True)
                # part A max
                ca = wk.tile([P, F], F32, tag="ca")
                nc.vector.tensor_scalar_mul(
                    out=ca[:], in0=conv_sb[:, b * F:(b + 1) * F],
                    scalar1=mA_sb[:, b:b + 1])
                trf = ps_tr.tile([P, P], F32, tag="trf")
                nc.tensor.transpose(trf[:], ca[:], ident_f[:])
                h4T = hTp.tile([P, P], F32, tag="h4T")
                nc.scalar.activation(h4T[:], trf[:], AF.Copy)
                nc.vector.tensor_reduce(out=bmaxA[:, b:b + 1], in_=h4T[:],
                                        axis=mybir.AxisListType.X, op=ALU.max)
                # part B max
                nc.vector.tensor_scalar_mul(
                    out=ca[:], in0=conv_sb[:, b * F:(b + 1) * F],
                    scalar1=mB_sb[:, b:b + 1])
                trf2 = ps_tr.tile([P, P], F32, tag="trf")
                nc.tensor.transpose(trf2[:], ca[:], ident_f[:])
                h4T2 = hTp.tile([P, P], F32, tag="h4T")
                nc.scalar.activation(h4T2[:], trf2[:], AF.Copy)
                nc.vector.tensor_reduce(out=bmaxB[:, b:b + 1], in_=h4T2[:],
                                        axis=mybir.AxisListType.X, op=ALU.max)
            # per-graph max via masked block-max over both parts
            gmaxT = ps.tile([P, 64], F32)
            GG = max(1, 512 // B)
            for g0 in range(0, G, GG):
                g1 = min(g0 + GG, G)
                w = (g1 - g0) * B
                mk_ps = ps_misc.tile([P, 512], F32, tag="misc")
                nc.tensor.matmul(mk_ps[:, 0:w], lhsT=ones_row[:],
                                 rhs=gmA_sb[:, g0 * B:g1 * B],
                                 start=True, stop=True)
                mck = wk.tile([P, GG * B], F32, tag="mck")
                nc.vector.tensor_tensor(
                    out=mck[:, 0:w],
                    in0=bmaxA[:].rearrange("p (o b) -> p o b", o=1)
                    .to_broadcast([P, g1 - g0, B]),
                    in1=mk_ps[:, 0:w].rearrange("p (g b) -> p g b", b=B),
                    op=ALU.mult)
                nc.vector.tensor_reduce(
                    out=gmaxT[:, g0:g1],
                    in_=mck[:, 0:w].rearrange("p (g b) -> p g b", b=B),
                    axis=mybir.AxisListType.X, op=ALU.max)
                # fold in part-B maxima
                mk_ps2 = ps_misc.tile([P, 512], F32, tag="misc")
                nc.tensor.matmul(mk_ps2[:, 0:w], lhsT=ones_row[:],
                                 rhs=gmB_sb[:, g0 * B:g1 * B],
                                 start=True, stop=True)
                mck2 = wk.tile([P, GG * B], F32, tag="mck2")
                nc.vector.tensor_tensor(
                    out=mck2[:, 0:w],
                    in0=bmaxB[:].rearrange("p (o b) -> p o b", o=1)
                    .to_broadcast([P, g1 - g0, B]),
                    in1=mk_ps2[:, 0:w].rearrange("p (g b) -> p g b", b=B),
                    op=ALU.mult)
                gB_red = wk.tile([P, 64], F32, tag="gBred")
                nc.vector.tensor_reduce(
                    out=gB_red[:, g0:g1],
                    in_=mck2[:, 0:w].rearrange("p (g b) -> p g b", b=B),
                    axis=mybir.AxisListType.X, op=ALU.max)
                nc.vector.tensor_tensor(
                    out=gmaxT[:, g0:g1], in0=gmaxT[:, g0:g1],
                    in1=gB_red[:, g0:g1], op=ALU.max)
            if G < 64:
                nc.vector.memset(gmaxT[:, G:64], 0.0)
            part = ps.tile([64, 2 * F + 1], F32)
            nc.scalar.activation(part[:, 0:F], pool_s[:, :], AF.Copy)
            nc.scalar.activation(part[:, 2 * F:2 * F + 1], pool_c[:, :],
                                 AF.Copy)
            gm_ps = ps_misc.tile([P, 512], F32, tag="misc")
            nc.tensor.transpose(gm_ps[0:64, 0:P], gmaxT[:], ident_f[:])
            nc.scalar.activation(part[:, F:2 * F], gm_ps[0:64, 0:P], AF.Copy)
            nc.sync.dma_start(out=pool_in[:, :], in_=part[:])
            nc.gpsimd.collective_compute(
                "AllGather", ALU.bypass,
                replica_groups=[list(range(NC))],
                ins=[pool_in[:, :].opt()],
                outs=[pool_out[:, :].opt()],
            )
            DP = 2 * F + 1
            pall = ps.tile([64, DP * NC], F32)
            nc.sync.dma_start(
                out=pall[:].rearrange("g (d c) -> g d c", c=NC),
                in_=pool_out[:, :].rearrange("(c g) d -> g d c", c=NC),
            )
            red = ps.tile([64, DP], F32)
            nc.vector.tensor_reduce(
                out=red[:, 0:F],
                in_=pall[:].rearrange("g (d c) -> g d c", c=NC)[:, 0:F, :],
                axis=mybir.AxisListType.X, op=ALU.add)
            nc.vector.tensor_reduce(
                out=red[:, 2 * F:2 * F + 1],
                in_=pall[:].rearrange("g (d c) -> g d c", c=NC)[:, 2 * F:2 * F + 1, :],
                axis=mybir.AxisListType.X, op=ALU.add)
            nc.vector.tensor_reduce(
                out=red[:, F:2 * F],
                in_=pall[:].rearrange("g (d c) -> g d c", c=NC)[:, F:2 * F, :],
                axis=mybir.AxisListType.X, op=ALU.max)
            rc = ps.tile([64, 1], F32)
            nc.vector.reciprocal(out=rc[:], in_=red[:, 2 * F:2 * F + 1])
            zmean = ps.tile([64, F], F32)
            nc.vector.tensor_scalar_mul(out=zmean[:], in0=red[:, 0:F],
                                        scalar1=rc[:])
            # ---------------- MLP ----------------
            lw1a = ps.tile([P, F], F32)
            nc.sync.dma_start(out=lw1a[:], in_=lw1_d[0:F, :])
            lw1b = ps.tile([P, F], F32)
            nc.sync.dma_start(out=lw1b[:], in_=lw1_d[F:2 * F, :])
            lw2_sb = ps.tile([P, 16], F32)
            nc.sync.dma_start(out=lw2_sb[:], in_=lw2_d[:, :])

            zTa_ps = ps_tr.tile([P, P], F32, tag="trf")
            nc.tensor.transpose(zTa_ps[:, 0:64], zmean[:], ident_f[0:64, 0:64])
            zTa = ps.tile([P, 64], F32)
            nc.scalar.activation(zTa[:], zTa_ps[:, 0:64], AF.Copy)
            zTb_ps = ps_tr.tile([P, P], F32, tag="trf")
            nc.tensor.transpose(zTb_ps[:, 0:64], red[:, F:2 * F],
                                ident_f[0:64, 0:64])
            zTb = ps.tile([P, 64], F32)
            nc.scalar.activation(zTb[:], zTb_ps[:, 0:64], AF.Copy)
            y1_ps = ps_misc.tile([P, 512], F32, tag="misc")
            nc.tensor.matmul(y1_ps[0:64, 0:F], lhsT=zTa[:], rhs=lw1a[:],
                             start=True, stop=False, skip_group_check=True)
            nc.tensor.matmul(y1_ps[0:64, 0:F], lhsT=zTb[:], rhs=lw1b[:],
                             start=False, stop=False, skip_group_check=True)
            nc.tensor.matmul(y1_ps[0:64, 0:F], lhsT=ones_row[:, 0:64],
                             rhs=lb1row[:], start=False, stop=True,
                             skip_group_check=True)
            y1 = ps.tile([64, F], F32)
            nc.scalar.activation(y1[:], y1_ps[0:64, 0:F], AF.Relu)
            yT_ps = ps_tr.tile([P, P], F32, tag="trf")
            nc.tensor.transpose(yT_ps[:, 0:64], y1[:], ident_f[0:64, 0:64])
            yT = ps.tile([P, 64], F32)
            nc.scalar.activation(yT[:], yT_ps[:, 0:64], AF.Copy)
            o_ps = ps_misc.tile([64, 16], F32, tag="misc")
            nc.tensor.matmul(o_ps[:, :], lhsT=yT[:], rhs=lw2_sb[:],
                             start=True, stop=False, skip_group_check=True)
            nc.tensor.matmul(o_ps[:, :], lhsT=ones_row[:, 0:64], rhs=lb2row[:],
                             start=False, stop=True, skip_group_check=True)
            o_sb = ps.tile([64, 16], F32)
            nc.scalar.activation(o_sb[:], o_ps[:, :], AF.Copy)
            nc.sync.dma_start(out=out_d[:, :], in_=o_sb[:])

    nc.compile()
    return nc


_CACHE = {}


def _get_program(key, cfg):
    if key not in _CACHE:
        _CACHE[key] = _build(cfg)
    return _CACHE[key]


def kernel(x, edge_index, batch, W1, b1, W2, b2, W3, b3, W4, b4,
           g1, be1, g2, be2, g3, be3, lw1, lb1, lw2, lb2):
    x = np.asarray(x)
    cfg, percore = _prep(x, edge_index, batch)
    C = int(lw2.shape[1])

    Wstack = np.stack([np.asarray(w, np.float32) for w in (W1, W2, W3, W4)]
                      ).astype(BF16NP)
    gam = np.stack([np.asarray(g, np.float32) for g in (g1, g2, g3)])
    bet = np.stack([np.asarray(b, np.float32) for b in (be1, be2, be3)])
    lw2p = np.zeros((lw2.shape[0], 16), np.float32)
    lw2p[:, :C] = np.asarray(lw2, np.float32)
    lb2p = np.zeros((1, 16), np.float32)
    lb2p[0, :C] = np.asarray(lb2, np.float32)

    shared = dict(
        W=Wstack,
        lw1=np.asarray(lw1, np.float32),
        lw2=lw2p,
        b4=np.asarray(b4, np.float32).reshape(1, -1),
        gamma=gam, beta=bet,
        lb1=np.asarray(lb1, np.float32).reshape(1, -1),
        lb2=lb2p,
    )
    in_maps = []
    for c in range(NC):
        m = {k: v[c] for k, v in percore.items()}
        m.update(shared)
        in_maps.append(m)

    key = (cfg["B"], cfg["K"], cfg["NTOT"], cfg["G"],
           tuple(cfg["regs"].reshape(-1).tolist()))
    nc = _get_program(key, cfg)

    global LAST_RESULTS
    if SIM:
        from concourse.bass_interp import MultiCoreSim
        sim = MultiCoreSim(nc, NC)
        for c in range(NC):
            for name, arr in in_maps[c].items():
                sim.cores[c].tensor(name)[:] = arr
        sim.simulate(check_with_hw=False)
        out = np.array(sim.cores[0].mem_tensor("out"))
        LAST_RESULTS = {"exec_time_ns": None}
        return out[:, :C].copy()

    from concourse import bass_utils
    if PROFILE:
        _install_ntff_hook_shim()
    res = bass_utils.run_bass_kernel_spmd(
        nc, in_maps, list(range(NC)), trace=PROFILE)
    LAST_RESULTS = {"exec_time_ns": res.exec_time_ns,
                    "mean_exec_time_ns": res.mean_exec_time_ns}
    return res.results[0]["out"][:, :C].copy()
